# revision 1
# baseline (speedup 1.0000x reference)
"""GroupQuantLinear on 8 Trainium2 NeuronCores.

y[b,s,o] = x[b,s,:] @ W[o,:] + bias[o], where W is dequantized on-device from
4-bit packed weights with per-(o, group) affine scale/bias (groups of 256 along
the 4096-wide input dim).

Sharding: tensor-parallel on out_features (8 shards of 2048 rows); x replicated.

Per-core kernel (Bass/Tile):
  Stage 1 (dequant): stream packed int32 words [o-tile 128, 1024 words],
    unpack 4 nibble planes with one fused DVE tensor_scalar (shift+and), then
    one fused DVE tensor_scalar (q * scale + wbias -> bf16) per (plane, group)
    with per-partition AP scalars.  Transpose the [o, in'] bf16 result to
    [in', o] via PE transposes, and store W^T into 4 DRAM quarter tensors.
  Stage 2 (matmul): composable_matmul_tile_kernel with kxm = x^T (f32 DMA +
    cast to bf16), kxn = streamed W^T quarters, fp32 PSUM accumulation, and the
    output bias folded into the PSUM->SBUF eviction (single DVE add).

Host marshalling is layout-only: x is transposed/permuted so the contraction
dim lands on SBUF partitions in the same nibble-plane-major order the on-chip
unpack produces (in' = plane*1024 + word, i.e. original index 4*word + plane).
"""

import numpy as np

B, S, IN, OUT, G = 2, 2048, 4096, 16384, 16
NCORES = 8
OSH = OUT // NCORES       # 2048 out rows per core
BS = B * S                # 4096
NW = IN // 4              # 1024 packed int32 words per out row
P = 128

_COMPILED = {}


def _build_nc():
    from contextlib import ExitStack

    import concourse.bass as bass
    import concourse.mybir as mybir
    import concourse.tile as tile
    from concourse import bacc
    from concourse.bass import ds, ts
    from concourse.masks import make_identity
    from concourse.kernels.tile_matmul import (
        ShapeInfo,
        cast_to_type,
        composable_matmul_tile_kernel,
        dma_from_dram_kxm,
        dma_to_dram_mxn,
    )

    f32 = mybir.dt.float32
    bf16 = mybir.dt.bfloat16
    i32 = mybir.dt.int32

    nc = bacc.Bacc(None, target_bir_lowering=False)

    xtp = nc.dram_tensor("xtp", [IN, BS], f32, kind="ExternalInput")
    wpk = nc.dram_tensor("wpk", [OSH, NW], i32, kind="ExternalInput")
    wsc = nc.dram_tensor("wsc", [OSH, G], f32, kind="ExternalInput")
    wbi = nc.dram_tensor("wbi", [OSH, G], f32, kind="ExternalInput")
    bias = nc.dram_tensor("bias", [1, OSH], f32, kind="ExternalInput")
    y = nc.dram_tensor("y", [BS, OSH], f32, kind="ExternalOutput")

    N_OT = OSH // P          # 16 o-tiles to dequantize
    N_KT = IN // 512         # 8 K tiles of 512
    GW = NW // G             # 64 words per group
    NWP = NW // P            # 8 in'-tiles per nibble plane
    NQ = OSH // 512          # 4 W^T quarter tensors

    with tile.TileContext(nc) as tc:
        with ExitStack() as ctx:
            const = ctx.enter_context(tc.tile_pool(name="const", bufs=1))
            dq = ctx.enter_context(tc.tile_pool(name="dq", bufs=2))
            dq_psum = ctx.enter_context(
                tc.tile_pool(name="dq_psum", bufs=2, space="PSUM")
            )
            dram = ctx.enter_context(tc.tile_pool(name="wt_dram", bufs=1, space="DRAM"))

            # ---- bias broadcast to [P, OSH] via K=1 fp32 matmuls ----
            bias_sb = const.tile([1, OSH], f32)
            nc.sync.dma_start(bias_sb[:], bias[:])
            ones_sb = const.tile([1, P], f32)
            nc.any.memset(ones_sb[:], 1.0)
            bias_bc = const.tile([P, OSH], f32)
            for j in range(OSH // 512):
                bps = dq_psum.tile([P, 512], f32, tag="biasps")
                nc.tensor.matmul(
                    bps[:], ones_sb[:], bias_sb[:, ts(j, 512)], start=True, stop=True
                )
                nc.any.tensor_copy(bias_bc[:, ts(j, 512)], bps[:])

            ident = const.tile([P, P], bf16)
            make_identity(nc, ident[:])

            # W^T quarters in DRAM: [IN, 512] each, rows in' plane-major order
            wt_q = [
                dram.tile([IN, 512], bf16, name=f"wt_q{i}") for i in range(NQ)
            ]

            # ---- Stage 1: dequant + transpose ----
            for ot in range(N_OT):
                osl = ts(ot, P)
                t_pk = dq.tile([P, NW], i32, tag="pk")
                nc.sync.dma_start(t_pk[:], wpk[osl, :])
                t_sc = dq.tile([P, G], f32, tag="sc")
                nc.sync.dma_start(t_sc[:], wsc[osl, :])
                t_bi = dq.tile([P, G], f32, tag="bi")
                nc.sync.dma_start(t_bi[:], wbi[osl, :])

                # wd[o, plane, w] bf16 == W'[o, in'] with in' = plane*NW + w
                wd = dq.tile([P, 4, NW], bf16, tag="wd")
                # unpack all 4 nibble planes (fused shift+and per plane)
                q4 = dq.tile([P, 4, NW], i32, tag="q4")
                for k in range(4):
                    nc.vector.tensor_scalar(
                        q4[:, k, :],
                        t_pk[:],
                        4 * k,
                        0xF,
                        mybir.AluOpType.logical_shift_right,
                        mybir.AluOpType.bitwise_and,
                    )
                # fused dequant, one DVE op per group across all 4 planes
                for g in range(G):
                    nc.vector.tensor_scalar(
                        wd[:, :, ts(g, GW)],
                        q4[:, :, ts(g, GW)],
                        t_sc[:, g : g + 1],
                        t_bi[:, g : g + 1],
                        mybir.AluOpType.mult,
                        mybir.AluOpType.add,
                    )

                # PE-transpose [o, in'] -> [in', o]; drain per K-tile of 512
                for kt in range(N_KT):
                    tps = dq_psum.tile([P, 4, P], bf16, tag="tps")
                    for s in range(4):
                        it = kt * 4 + s  # global in'-tile index
                        nc.tensor.transpose(
                            tps[:, s, :],
                            wd[:, it // NWP, ts(it % NWP, P)],
                            ident[:],
                        )
                    stg = dq.tile([P, 4, P], bf16, tag="stg")
                    nc.any.tensor_copy(stg[:], tps[:])
                    dst = wt_q[ot // 4].rearrange(
                        "(kt s p) c -> p kt s c", p=P, s=4
                    )[:, kt, :, ts(ot % 4, P)]
                    nc.sync.dma_start(dst, stg[:])

            # ---- Stage 2: matmul y = x @ W^T + bias ----
            kxm_pool = ctx.enter_context(tc.tile_pool(name="kxm", bufs=3))
            kxm_cast = ctx.enter_context(tc.tile_pool(name="kxmc", bufs=9))
            kxn_pool = ctx.enter_context(tc.tile_pool(name="kxn", bufs=9))

            kxm_producer, kxm_shape = dma_from_dram_kxm(kxm_pool, xtp[:])
            kxm_producer = cast_to_type(kxm_producer, kxm_cast, bf16)

            kxn_shape = ShapeInfo(pdims=((P, IN // P),), fdims=(OSH,))

            def kxn_producer(nc_, md):
                t = kxn_pool.tile([P, md.k_subtiles, md.n_tile], bf16, tag="kxn")
                src = wt_q[md.n_tile_idx].rearrange(
                    "(kt s p) c -> p kt s c", p=P, s=4
                )[:, md.k_tile_idx, :, :]
                nc_.sync.dma_start(t[:], src)
                return t

            def bias_evict(nc_, psum, sbuf, md):
                start = md.n_tile_idx * md.n_tile + md.n_subtile_idx * md.n_subtile
                nc_.vector.tensor_add(
                    sbuf, psum, bias_bc[:, ds(start, md.n_subtile)]
                )

            composable_matmul_tile_kernel(
                tc,
                kxm_shape=kxm_shape,
                kxn_shape=kxn_shape,
                output_type=f32,
                kxm_producer=kxm_producer,
                kxn_producer=kxn_producer,
                mxn_consumer=dma_to_dram_mxn(y[:]),
                mxn_subtile_reducer=bias_evict,
                psum_n_bufs=1,
                temps_n_bufs=2,
            )

    nc.compile()
    return nc


def _get_compiled():
    if "nc" not in _COMPILED:
        _COMPILED["nc"] = _build_nc()
    return _COMPILED["nc"]


def _marshal(input, w_packed, w_scale, w_bias, bias):
    x = np.ascontiguousarray(input, dtype=np.float32).reshape(BS, IN)
    # x^T with rows permuted to plane-major in' order: in' = k*NW + w <- 4w + k
    xt = x.T  # [IN, BS]
    xtp = np.ascontiguousarray(
        xt.reshape(NW, 4, BS).transpose(1, 0, 2).reshape(IN, BS)
    )
    in_maps = []
    for c in range(NCORES):
        osl = slice(c * OSH, (c + 1) * OSH)
        in_maps.append(
            {
                "xtp": xtp,
                "wpk": np.ascontiguousarray(w_packed[osl].reshape(OSH, NW)),
                "wsc": np.ascontiguousarray(w_scale[osl].reshape(OSH, G)),
                "wbi": np.ascontiguousarray(w_bias[osl].reshape(OSH, G)),
                "bias": np.ascontiguousarray(bias[osl].reshape(1, OSH)),
            }
        )
    return in_maps


def kernel(input, w_packed, w_scale, w_bias, bias, _trace=False, _trace_kwargs=None):
    from concourse.bass_utils import run_bass_kernel_spmd

    nc = _get_compiled()
    in_maps = _marshal(input, w_packed, w_scale, w_bias, bias)
    res = run_bass_kernel_spmd(
        nc,
        in_maps,
        core_ids=list(range(NCORES)),
        trace=_trace,
        **(_trace_kwargs or {}),
    )
    ys = [res.results[c]["y"] for c in range(NCORES)]
    out = np.concatenate(ys, axis=1).reshape(B, S, OUT).astype(np.float32)
    if _trace:
        return out, res
    return out

